# revision 29
# baseline (speedup 1.0000x reference)
"""Trainium2 Bass kernel for nn_ControEncodNet (gnn_message_passing).

Computation (reference):
    feats = sigmoid(tanh(tanh(F_ @ X @ W1 + b1) @ W2 + b2) @ W3 + b3)   [N, 64]
    d2    = ||feats_i - feats_j||^2 ;  dist = sqrt(max(d2, 1e-12))
    out   = (dist * (1 - eye) * path_forces) @ members                   [N, 16]

Sharding: rows (nodes) split across 8 cores, 1024 rows each. Each core
computes the MLP embedding for its row block, all-gathers the [64, 1024]
featsT blocks, then computes its block of the masked pairwise-distance
matrix fully locally and contracts with `members`.

Numerics notes (this problem is badly conditioned):
  - F_@X is tiny (~1e-2), so feats spread across nodes is only ~1.5e-4
    around 0.5.  sq_i + sq_j - 2*G cancels catastrophically (true d2
    ~1e-6 vs terms ~40).  We therefore CENTER the features by their
    global mean before the distance matmul: d2 is translation invariant
    and the centered terms are all ~1e-6, giving ~1e-10 accuracy where
    the fp32 reference itself is only ~1e-6-accurate on d2.
  - The feats output must be fp32-quality, so F_@X uses a 3-term
    bf16-split matmul (hi*hi + hi*lo + lo*hi ~ 2^-18 relative) and the
    small MLP matmuls run in plain fp32.  The big distance/contraction
    matmuls use fp32r (fast PE path, ~2^-12) which is ample there.

Device-side layouts are all transposed ([feature, node]) so every matmul
contracts along the SBUF partition dim with no on-device transposes:
  - F_ enters as F_[rows,:].T (host pre-transpose, bf16 hi/lo pair)
  - path_forces enters as (pf^2 with zeroed diagonal)[rows,:].T
    (pf^2 lets dist*pf = sqrt(max(d2,eps)*pf^2) fuse clamp+mask+mult:
     one DVE scalar_tensor_tensor + one ACT Sqrt per tile)
  - d2 comes from one augmented matmul per tile:
     d2[j,i] = sum_k augL[k,j]*augR[k,i],
     augL = [-2*cfeat_all; ones; csq_all], augR = [cfeat_loc; csq_loc; ones]
"""

import ml_dtypes
import numpy as np

import concourse.bass as bass
import concourse.bacc as bacc
import concourse.mybir as mybir
import concourse.tile as tile
from concourse import bass_utils

AF = mybir.ActivationFunctionType
ALU = mybir.AluOpType

N = 8192
NCORES = 8
BLK = N // NCORES  # 1024 rows per core
IN_F = 128
H1, H2, H3 = 256, 128, 64
NCLUST = 16
P = 128  # SBUF partitions
IC = 512  # i-chunk (free dim per matmul)
N_IC = BLK // IC  # 2
N_K = N // P  # 64 contraction / j chunks

F32 = mybir.dt.float32
F32R = mybir.dt.float32r
BF16 = mybir.dt.bfloat16

PF_DT = BF16


def _emit(tc, nc, T):
    ctxpools = []

    def pool(name, bufs, space="SBUF"):
        p = tc.tile_pool(name=name, bufs=bufs, space=space)
        ctxpools.append(p)
        return p.__enter__()

    const = pool("const", 1)
    ftp = pool("ftp", 4)
    pfp = pool("pfp", 10)
    mlp = pool("mlp", 2)
    fsqp = pool("fsqp", 4)
    gp = pool("gp", 3)
    fwork = pool("fwork", 5)
    pacc = pool("pacc", 2, space="PSUM")
    ptrans = pool("ptrans", 4, space="PSUM")
    psqp = pool("psqp", 2, space="PSUM")
    dram = pool("dram", 1, space="DRAM")

    # ---------------- phase-1 head start: first F tiles before constants ----
    early_fthl = []
    for kp in range(2):
        fthl2 = ftp.tile([P, 4 * BLK], BF16, tag="fthl", name=f"fthl_e{kp}")
        nc.sync.dma_start(
            fthl2[:].rearrange("p (t c) -> p t c", c=2 * BLK),
            T["fthl"][2 * kp * P : (2 * kp + 2) * P, :].rearrange(
                "(t p) c -> p t c", p=P
            ),
        )
        early_fthl.append(fthl2)

    # ---------------- constants ----------------
    # X split + members arrive host-pre-arranged ([p, k*c] / [p, k*m]) so the
    # DMA moves contiguous 16KB-per-partition rows instead of 256B packets
    xwh = const.tile([P, N_K * IN_F], BF16)
    nc.sync.dma_start(xwh[:], T["xhi"][:])
    xwl = const.tile([P, N_K * IN_F], BF16)
    nc.sync.dma_start(xwl[:], T["xlo"][:])
    memsb = const.tile([P, N_K * NCLUST], F32R)
    nc.sync.dma_start(memsb[:], T["members"][:])
    w1sb = const.tile([P, H1], F32)
    nc.sync.dma_start(w1sb[:], T["w1"][:])
    w2sb = const.tile([P, H1], F32)  # w2sb[p, t*128+m] = W2[t*128+p, m]
    nc.sync.dma_start(
        w2sb[:].rearrange("p (t m) -> p t m", m=H2),
        T["w2"].rearrange("(t p) m -> p t m", p=P),
    )
    w3sb = const.tile([P, H3], F32)
    nc.sync.dma_start(w3sb[:], T["w3"][:])
    b1sb = const.tile([P, 2], F32)
    nc.sync.dma_start(b1sb[:], T["b1"].rearrange("(t p) -> p t", p=P))
    b2sb = const.tile([P, 1], F32)
    nc.sync.dma_start(b2sb[:], T["b2"].rearrange("(o p) -> p o", o=1))
    b3sb = const.tile([H3, 1], F32)
    nc.sync.dma_start(b3sb[:], T["b3"].rearrange("(o p) -> p o", o=1))
    ones64 = const.tile([H3, 1], F32R)
    nc.sync.dma_start(ones64[:], T["cones"][:, 0:1])

    featsF32 = const.tile([H3, BLK], F32)  # raw fp32 feats (output only)
    resid16 = const.tile([H3, BLK], BF16)  # feats - fref (AG payload)
    # aug matrices for the pairwise-distance matmul (centered features)
    augR = const.tile([H3 + 2, BLK], F32R)  # [cfeat_loc; csq_loc; ones]
    augLs = [
        const.tile([H3 + 2, BLK], F32R, name=f"augL{r}", tag=f"augL{r}")
        for r in range(NCORES)
    ]  # per-rank blocks of [-2*cfeat_all; ones; csq_all]
    # fref = MLP(0): identical on every core; distances are shift-invariant,
    # and feats-fref is ~1e-4 so the d2 matmul terms stay ~1e-6 (no
    # catastrophic cancellation and no global mean needed).
    zin = const.tile([P, 1], F32)
    nc.vector.memset(zin[:], 0.0)
    h1ref = const.tile([P, 2], F32)
    h2ref = const.tile([P, 1], F32)
    fref = const.tile([H3, 1], F32)

    # ---------------- phase 1: inT = (F_blk @ X).T via 3-term bf16 split ----------------
    pin0 = pacc.tile([P, IC], F32, tag="acc")
    pin1 = pacc.tile([P, IC], F32, tag="acc")
    pins = [pin0, pin1]
    for kp in range(N_K // 2):
        if kp < 2:
            fthl2 = early_fthl[kp]
        else:
            fthl2 = ftp.tile([P, 4 * BLK], BF16, tag="fthl")
            nc.sync.dma_start(
                fthl2[:].rearrange("p (t c) -> p t c", c=2 * BLK),
                T["fthl"][2 * kp * P : (2 * kp + 2) * P, :].rearrange(
                    "(t p) c -> p t c", p=P
                ),
            )
        for kh in range(2):
            k = 2 * kp + kh
            ksl = slice(k * IN_F, (k + 1) * IN_F)
            fth = fthl2[:, kh * 2 * BLK : kh * 2 * BLK + BLK]
            ftl = fthl2[:, kh * 2 * BLK + BLK : (kh + 1) * 2 * BLK]
            for ic in range(N_IC):
                isl = slice(ic * IC, (ic + 1) * IC)
                nc.tensor.matmul(
                    pins[ic][:], lhsT=xwh[:, ksl], rhs=fth[:, isl],
                    start=(k == 0), stop=False,
                )
                nc.tensor.matmul(
                    pins[ic][:], lhsT=xwl[:, ksl], rhs=fth[:, isl],
                    start=False, stop=False,
                )
                nc.tensor.matmul(
                    pins[ic][:], lhsT=xwh[:, ksl], rhs=ftl[:, isl],
                    start=False, stop=(k == N_K - 1),
                )

    # ---------------- phase 1b: MLP in plain fp32 (transposed activations) ----------------
    for ic in range(N_IC):
        isl = slice(ic * IC, (ic + 1) * IC)
        int_sb = mlp.tile([P, IC], F32, tag="int")
        nc.scalar.activation(int_sb[:], pins[ic][:], AF.Copy)
        h1a = mlp.tile([P, IC], F32, tag="h1a")
        h1b = mlp.tile([P, IC], F32, tag="h1b")
        for t, h1t in enumerate((h1a, h1b)):
            ph = ptrans.tile([P, IC], F32, tag="tr")
            nc.tensor.matmul(
                ph[:], lhsT=w1sb[:, t * H2 : (t + 1) * H2], rhs=int_sb[:],
                start=True, stop=True,
            )
            nc.scalar.activation(h1t[:], ph[:], AF.Tanh, bias=b1sb[:, t : t + 1])
        ph2 = ptrans.tile([P, IC], F32, tag="tr")
        nc.tensor.matmul(ph2[:], lhsT=w2sb[:, 0:H2], rhs=h1a[:], start=True, stop=False)
        nc.tensor.matmul(
            ph2[:], lhsT=w2sb[:, H2 : 2 * H2], rhs=h1b[:], start=False, stop=True
        )
        h2sb = mlp.tile([P, IC], F32, tag="h2")
        nc.scalar.activation(h2sb[:], ph2[:], AF.Tanh, bias=b2sb[:, 0:1])
        pf_ = ptrans.tile([H3, IC], F32, tag="tr")
        nc.tensor.matmul(pf_[:], lhsT=w3sb[:], rhs=h2sb[:], start=True, stop=True)
        nc.scalar.activation(featsF32[0:H3, isl], pf_[:], AF.Sigmoid, bias=b3sb[:, 0:1])

    # fref = MLP(0) via [*,1] column pipeline (tiny)
    for t in range(2):
        nc.scalar.activation(h1ref[:, t : t + 1], zin[:], AF.Tanh, bias=b1sb[:, t : t + 1])
    ph2r = ptrans.tile([P, 1], F32, tag="tr")
    nc.tensor.matmul(ph2r[:], lhsT=w2sb[:, 0:H2], rhs=h1ref[:, 0:1], start=True, stop=False)
    nc.tensor.matmul(ph2r[:], lhsT=w2sb[:, H2 : 2 * H2], rhs=h1ref[:, 1:2], start=False, stop=True)
    nc.scalar.activation(h2ref[:], ph2r[:], AF.Tanh, bias=b2sb[:, 0:1])
    pfr = ptrans.tile([H3, 1], F32, tag="tr")
    nc.tensor.matmul(pfr[:], lhsT=w3sb[:], rhs=h2ref[:], start=True, stop=True)
    nc.scalar.activation(fref[:], pfr[:], AF.Sigmoid, bias=b3sb[:, 0:1])

    # feats output (transposed block, full fp32)
    nc.sync.dma_start(T["featsT"][:], featsF32[:])
    # bf16 residuals: the AG payload (quarters collective bytes vs fp32;
    # ~6e-7 abs rounding on a ~3e-4 signal washes out in the 8192-term sum)
    nc.vector.tensor_scalar(resid16[:], featsF32[:], fref[:], None, op0=ALU.subtract)

    # ---------------- all-gather featsT blocks (fp32) ----------------
    agin = dram.tile([H3, BLK], BF16)
    agout = dram.tile([H3 * NCORES, BLK], BF16, addr_space="Shared")
    nc.sync.dma_start(agin[:], resid16[:])
    nc.gpsimd.collective_compute(
        "AllGather",
        ALU.bypass,
        replica_groups=[list(range(NCORES))],
        ins=[agin.opt()],
        outs=[agout.opt()],
    )

    # centered local features -> augR rows 0..63 (fp32 -> fp32r rounding)
    nc.vector.tensor_scalar(
        augR[0:H3, :], featsF32[:], fref[:], None, op0=ALU.subtract
    )
    # csq_loc at partition 64 (legal compute-engine partition base)
    for ic in range(N_IC):
        isl = slice(ic * IC, (ic + 1) * IC)
        fsqr = fsqp.tile([H3, IC], F32R, tag="fsq")
        nc.scalar.activation(fsqr[:], augR[0:H3, isl], AF.Square)
        psq = psqp.tile([1, IC], F32, tag="sq")
        nc.tensor.matmul(psq[:], lhsT=ones64[:], rhs=fsqr[:], start=True, stop=True)
        nc.scalar.activation(augR[H3 : H3 + 1, isl], psq[:], AF.Copy)
    # ones at partition 65: compute engines can't address base 65; DMA can
    nc.sync.dma_start(augR[H3 + 1 : H3 + 2, :], T["cones"][0:1, :])

    # per-rank augL blocks: rows 0..63 = -2*resid; row 64 = ones; row 65 = csq.
    # Separate tiles keep the d2 matmuls for rank r unblocked as soon as
    # block r is assembled (no false dependency on later blocks).
    for r in range(NCORES):
        aL = augLs[r]
        gblk = gp.tile([H3, BLK], BF16, tag="gblk")
        nc.sync.dma_start(gblk[:], agout[H3 * r : H3 * (r + 1), :])
        nc.vector.tensor_scalar_mul(aL[0:H3, :], gblk[:], -2.0)
        nc.sync.dma_start(aL[H3 : H3 + 1, :], T["cones"][0:1, :])
        for g in range(N_IC):
            gsl = slice(g * IC, (g + 1) * IC)
            fsql = fsqp.tile([H3, IC], F32R, tag="fsq")
            nc.scalar.activation(fsql[:], aL[0:H3, gsl], AF.Square)  # = 4*cfeat^2
            psql = psqp.tile([1, IC], F32, tag="sq")
            nc.tensor.matmul(psql[:], lhsT=ones64[:], rhs=fsql[:], start=True, stop=True)
            sqst = fsqp.tile([1, IC], F32R, tag="fsq")
            nc.scalar.activation(sqst[:], psql[:], AF.Copy, scale=0.25)
            nc.sync.dma_start(aL[H3 + 1 : H3 + 2, gsl], sqst[:])

    # ---------------- phase 2: dist block, contract with members ----------------
    pout0 = pacc.tile([NCLUST, IC], F32, tag="acc")
    pout1 = pacc.tile([NCLUST, IC], F32, tag="acc")
    pouts = [pout0, pout1]
    # dist = sqrt(d2 + 2e-10): the bias keeps the (centered, ~1e-11-noise)
    # diagonal non-negative, replacing the reference's max(d2, 1e-12) clamp
    # with O(1e-4) relative effect only on the closest pair's distance.
    for jp in range(N_K // 2):
        pft2 = pfp.tile([P, 2 * BLK], PF_DT, tag="pft")
        nc.sync.dma_start(
            pft2[:].rearrange("p (t c) -> p t c", c=BLK),
            T["pf2t"][2 * jp * P : (2 * jp + 2) * P, :].rearrange(
                "(t p) c -> p t c", p=P
            ),
        )
        for jh in range(2):
            jc = 2 * jp + jh
            for ic in range(N_IC):
                isl = slice(ic * IC, (ic + 1) * IC)
                pd2 = ptrans.tile([P, IC], F32, tag="tr")
                nc.tensor.matmul(
                    pd2[:],
                    lhsT=augLs[jc // 8][:, (jc % 8) * P : (jc % 8 + 1) * P],
                    rhs=augR[:, isl],
                    start=True,
                    stop=True,
                )
                tt = fwork.tile([P, IC], F32, tag="tt")
                nc.vector.scalar_tensor_tensor(
                    tt[:],
                    pd2[:],
                    1e-12,
                    pft2[:, jh * BLK + ic * IC : jh * BLK + (ic + 1) * IC],
                    op0=ALU.max,
                    op1=ALU.mult,
                )
                fo = fwork.tile([P, IC], F32R, tag="fo")
                nc.scalar.activation(fo[:], tt[:], AF.Sqrt)
                nc.tensor.matmul(
                    pouts[ic][:],
                    lhsT=memsb[:, jc * NCLUST : (jc + 1) * NCLUST],
                    rhs=fo[:],
                    start=(jc == 0),
                    stop=(jc == N_K - 1),
                )
    for ic in range(N_IC):
        osb = fwork.tile([NCLUST, IC], F32, tag="tt")
        nc.vector.tensor_copy(osb[:], pouts[ic][:])
        nc.sync.dma_start(T["outT"][:, ic * IC : (ic + 1) * IC], osb[:])

    for p in reversed(ctxpools):
        p.__exit__(None, None, None)


def _build():
    nc = bacc.Bacc(
        "TRN2",
        target_bir_lowering=False,
        debug=False,
        enable_asserts=False,
        num_devices=NCORES,
    )
    T = {}
    T["fthl"] = nc.dram_tensor("fthl", [N, 2 * BLK], BF16, kind="ExternalInput").ap()
    T["xhi"] = nc.dram_tensor("xhi", [P, N_K * IN_F], BF16, kind="ExternalInput").ap()
    T["xlo"] = nc.dram_tensor("xlo", [P, N_K * IN_F], BF16, kind="ExternalInput").ap()
    T["pf2t"] = nc.dram_tensor("pf2t", [N, BLK], PF_DT, kind="ExternalInput").ap()
    T["members"] = nc.dram_tensor("members", [P, N_K * NCLUST], F32R, kind="ExternalInput").ap()
    T["w1"] = nc.dram_tensor("w1", [IN_F, H1], F32, kind="ExternalInput").ap()
    T["b1"] = nc.dram_tensor("b1", [H1], F32, kind="ExternalInput").ap()
    T["w2"] = nc.dram_tensor("w2", [H1, H2], F32, kind="ExternalInput").ap()
    T["b2"] = nc.dram_tensor("b2", [H2], F32, kind="ExternalInput").ap()
    T["w3"] = nc.dram_tensor("w3", [H2, H3], F32, kind="ExternalInput").ap()
    T["b3"] = nc.dram_tensor("b3", [H3], F32, kind="ExternalInput").ap()
    T["cones"] = nc.dram_tensor("cones", [H3, BLK], F32R, kind="ExternalInput").ap()
    T["featsT"] = nc.dram_tensor("featsT", [H3, BLK], F32, kind="ExternalOutput").ap()
    T["outT"] = nc.dram_tensor("outT", [NCLUST, BLK], F32, kind="ExternalOutput").ap()
    with tile.TileContext(nc) as tc:
        _emit(tc, nc, T)
    nc.compile()
    return nc


_NC = None
_ONES = np.ones((H3, BLK), np.float32)
_PF_NP = mybir.dt.np(PF_DT)


def _get_nc():
    global _NC
    if _NC is None:
        _NC = _build()
    return _NC


def kernel(F_, X, path_forces, members, W1, b1, W2, b2, W3, b3,
           _trace=False, _return_raw=False, _tmpdir=None):
    F_ = np.asarray(F_, np.float32)
    X = np.asarray(X, np.float32)
    path_forces = np.asarray(path_forces, np.float32)
    members = np.asarray(members, np.float32)
    W1 = np.asarray(W1, np.float32)
    b1 = np.asarray(b1, np.float32)
    W2 = np.asarray(W2, np.float32)
    b2 = np.asarray(b2, np.float32)
    W3 = np.asarray(W3, np.float32)
    b3 = np.asarray(b3, np.float32)

    nc = _get_nc()

    bf = ml_dtypes.bfloat16

    def _prearrange(a):  # [64*128, C] -> [128, 64*C]
        c = a.shape[1]
        return np.ascontiguousarray(
            a.reshape(N_K, P, c).transpose(1, 0, 2).reshape(P, N_K * c)
        )

    xhi32 = X.astype(bf).astype(np.float32)
    xhi = _prearrange(X.astype(bf).astype(np.float32)).astype(bf)
    xlo = _prearrange(X - xhi32).astype(bf)
    members_pa = _prearrange(members)
    pf2 = path_forces * path_forces
    np.fill_diagonal(pf2, 0.0)

    in_maps = []
    for c in range(NCORES):
        sl = slice(c * BLK, (c + 1) * BLK)
        ftT = np.ascontiguousarray(F_[sl, :].T)
        fthi = ftT.astype(bf)
        ftlo = (ftT - fthi.astype(np.float32)).astype(bf)
        fthl = np.concatenate([fthi, ftlo], axis=1)
        in_maps.append(
            {
                "fthl": fthl,
                "xhi": xhi,
                "xlo": xlo,
                "pf2t": np.ascontiguousarray(pf2[sl, :].T).astype(_PF_NP),
                "members": members_pa,
                "w1": W1,
                "b1": b1,
                "w2": W2,
                "b2": b2,
                "w3": W3,
                "b3": b3,
                "cones": _ONES,
            }
        )

    res = bass_utils.run_bass_kernel_spmd(
        nc, in_maps, core_ids=list(range(NCORES)), trace=_trace, tmpdir=_tmpdir
    )
    feats = np.concatenate(
        [np.asarray(res.results[c]["featsT"]).T for c in range(NCORES)], axis=0
    )
    out = np.concatenate(
        [np.asarray(res.results[c]["outT"]).T for c in range(NCORES)], axis=0
    )
    if _return_raw:
        return (feats, out), res
    return feats, out


# revision 30
# speedup vs baseline: 1.0187x; 1.0187x over previous
"""Trainium2 Bass kernel for nn_ControEncodNet (gnn_message_passing).

Computation (reference):
    feats = sigmoid(tanh(tanh(F_ @ X @ W1 + b1) @ W2 + b2) @ W3 + b3)   [N, 64]
    d2    = ||feats_i - feats_j||^2 ;  dist = sqrt(max(d2, 1e-12))
    out   = (dist * (1 - eye) * path_forces) @ members                   [N, 16]

Sharding: rows (nodes) split across 8 cores, 1024 rows each. Each core
computes the MLP embedding for its row block, all-gathers the [64, 1024]
featsT blocks, then computes its block of the masked pairwise-distance
matrix fully locally and contracts with `members`.

Numerics notes (this problem is badly conditioned):
  - F_@X is tiny (~1e-2), so feats spread across nodes is only ~1.5e-4
    around 0.5.  sq_i + sq_j - 2*G cancels catastrophically (true d2
    ~1e-6 vs terms ~40).  We therefore CENTER the features by their
    global mean before the distance matmul: d2 is translation invariant
    and the centered terms are all ~1e-6, giving ~1e-10 accuracy where
    the fp32 reference itself is only ~1e-6-accurate on d2.
  - The feats output must be fp32-quality, so F_@X uses a 3-term
    bf16-split matmul (hi*hi + hi*lo + lo*hi ~ 2^-18 relative) and the
    small MLP matmuls run in plain fp32.  The big distance/contraction
    matmuls use fp32r (fast PE path, ~2^-12) which is ample there.

Device-side layouts are all transposed ([feature, node]) so every matmul
contracts along the SBUF partition dim with no on-device transposes:
  - F_ enters as F_[rows,:].T (host pre-transpose, bf16 hi/lo pair)
  - path_forces enters as (pf^2 with zeroed diagonal)[rows,:].T
    (pf^2 lets dist*pf = sqrt(max(d2,eps)*pf^2) fuse clamp+mask+mult:
     one DVE scalar_tensor_tensor + one ACT Sqrt per tile)
  - d2 comes from one augmented matmul per tile:
     d2[j,i] = sum_k augL[k,j]*augR[k,i],
     augL = [-2*cfeat_all; ones; csq_all], augR = [cfeat_loc; csq_loc; ones]
"""

import ml_dtypes
import numpy as np

import concourse.bass as bass
import concourse.bacc as bacc
import concourse.mybir as mybir
import concourse.tile as tile
from concourse import bass_utils

AF = mybir.ActivationFunctionType
ALU = mybir.AluOpType

N = 8192
NCORES = 8
BLK = N // NCORES  # 1024 rows per core
IN_F = 128
H1, H2, H3 = 256, 128, 64
NCLUST = 16
P = 128  # SBUF partitions
IC = 512  # i-chunk (free dim per matmul)
N_IC = BLK // IC  # 2
N_K = N // P  # 64 contraction / j chunks

F32 = mybir.dt.float32
F32R = mybir.dt.float32r
BF16 = mybir.dt.bfloat16

PF_DT = BF16


def _emit(tc, nc, T):
    ctxpools = []

    def pool(name, bufs, space="SBUF"):
        p = tc.tile_pool(name=name, bufs=bufs, space=space)
        ctxpools.append(p)
        return p.__enter__()

    const = pool("const", 1)
    ftp = pool("ftp", 4)
    pfp = pool("pfp", 10)
    mlp = pool("mlp", 2)
    fsqp = pool("fsqp", 4)
    gp = pool("gp", 3)
    fwork = pool("fwork", 5)
    pacc = pool("pacc", 2, space="PSUM")
    ptrans = pool("ptrans", 4, space="PSUM")
    psqp = pool("psqp", 2, space="PSUM")
    dram = pool("dram", 1, space="DRAM")

    # ---------------- phase-1 head start: first F tiles before constants ----
    early_fthl = []
    for kp in range(2):
        fthl2 = ftp.tile([P, 4 * BLK], BF16, tag="fthl", name=f"fthl_e{kp}")
        for kh in range(2):
            nc.sync.dma_start(
                fthl2[:, kh * 2 * BLK : (kh + 1) * 2 * BLK],
                T["fthl"][(2 * kp + kh) * P : (2 * kp + kh + 1) * P, :],
            )
        early_fthl.append(fthl2)

    # ---------------- constants ----------------
    # X split + members arrive host-pre-arranged ([p, k*c] / [p, k*m]) so the
    # DMA moves contiguous 16KB-per-partition rows instead of 256B packets
    xwh = const.tile([P, N_K * IN_F], BF16)
    xwl = const.tile([P, N_K * IN_F], BF16)
    nc.sync.dma_start(xwh[:, 0:IN_F], T["xhi"][:, 0:IN_F])
    nc.sync.dma_start(xwl[:, 0:IN_F], T["xlo"][:, 0:IN_F])
    nc.sync.dma_start(xwh[:, IN_F:], T["xhi"][:, IN_F:])
    nc.sync.dma_start(xwl[:, IN_F:], T["xlo"][:, IN_F:])
    memsb = const.tile([P, N_K * NCLUST], F32R)
    nc.sync.dma_start(memsb[:], T["members"][:])
    w1sb = const.tile([P, H1], F32)
    nc.sync.dma_start(w1sb[:], T["w1"][:])
    w2sb = const.tile([P, H1], F32)  # w2sb[p, t*128+m] = W2[t*128+p, m]
    nc.sync.dma_start(
        w2sb[:].rearrange("p (t m) -> p t m", m=H2),
        T["w2"].rearrange("(t p) m -> p t m", p=P),
    )
    w3sb = const.tile([P, H3], F32)
    nc.sync.dma_start(w3sb[:], T["w3"][:])
    b1sb = const.tile([P, 2], F32)
    nc.sync.dma_start(b1sb[:], T["b1"].rearrange("(t p) -> p t", p=P))
    b2sb = const.tile([P, 1], F32)
    nc.sync.dma_start(b2sb[:], T["b2"].rearrange("(o p) -> p o", o=1))
    b3sb = const.tile([H3, 1], F32)
    nc.sync.dma_start(b3sb[:], T["b3"].rearrange("(o p) -> p o", o=1))
    ones64 = const.tile([H3, 1], F32R)
    nc.sync.dma_start(ones64[:], T["cones"][:, 0:1])

    featsF32 = const.tile([H3, BLK], F32)  # raw fp32 feats (output only)
    resid16 = const.tile([H3, BLK], BF16)  # feats - fref (AG payload)
    # aug matrices for the pairwise-distance matmul (centered features)
    augR = const.tile([H3 + 2, BLK], F32R)  # [cfeat_loc; csq_loc; ones]
    augLs = [
        const.tile([H3 + 2, BLK], F32R, name=f"augL{r}", tag=f"augL{r}")
        for r in range(NCORES)
    ]  # per-rank blocks of [-2*cfeat_all; ones; csq_all]
    # fref = MLP(0): identical on every core; distances are shift-invariant,
    # and feats-fref is ~1e-4 so the d2 matmul terms stay ~1e-6 (no
    # catastrophic cancellation and no global mean needed).
    zin = const.tile([P, 1], F32)
    nc.vector.memset(zin[:], 0.0)
    h1ref = const.tile([P, 2], F32)
    h2ref = const.tile([P, 1], F32)
    fref = const.tile([H3, 1], F32)

    # ---------------- phase 1: inT = (F_blk @ X).T via 3-term bf16 split ----------------
    pin0 = pacc.tile([P, IC], F32, tag="acc")
    pin1 = pacc.tile([P, IC], F32, tag="acc")
    pins = [pin0, pin1]
    for kp in range(N_K // 2):
        if kp < 2:
            fthl2 = early_fthl[kp]
        else:
            fthl2 = ftp.tile([P, 4 * BLK], BF16, tag="fthl")
            nc.sync.dma_start(
                fthl2[:].rearrange("p (t c) -> p t c", c=2 * BLK),
                T["fthl"][2 * kp * P : (2 * kp + 2) * P, :].rearrange(
                    "(t p) c -> p t c", p=P
                ),
            )
        for kh in range(2):
            k = 2 * kp + kh
            ksl = slice(k * IN_F, (k + 1) * IN_F)
            fth = fthl2[:, kh * 2 * BLK : kh * 2 * BLK + BLK]
            ftl = fthl2[:, kh * 2 * BLK + BLK : (kh + 1) * 2 * BLK]
            for ic in range(N_IC):
                isl = slice(ic * IC, (ic + 1) * IC)
                nc.tensor.matmul(
                    pins[ic][:], lhsT=xwh[:, ksl], rhs=fth[:, isl],
                    start=(k == 0), stop=False,
                )
                nc.tensor.matmul(
                    pins[ic][:], lhsT=xwl[:, ksl], rhs=fth[:, isl],
                    start=False, stop=False,
                )
                nc.tensor.matmul(
                    pins[ic][:], lhsT=xwh[:, ksl], rhs=ftl[:, isl],
                    start=False, stop=(k == N_K - 1),
                )

    # ---------------- phase 1b: MLP in plain fp32 (transposed activations) ----------------
    for ic in range(N_IC):
        isl = slice(ic * IC, (ic + 1) * IC)
        int_sb = mlp.tile([P, IC], F32, tag="int")
        nc.scalar.activation(int_sb[:], pins[ic][:], AF.Copy)
        h1a = mlp.tile([P, IC], F32, tag="h1a")
        h1b = mlp.tile([P, IC], F32, tag="h1b")
        for t, h1t in enumerate((h1a, h1b)):
            ph = ptrans.tile([P, IC], F32, tag="tr")
            nc.tensor.matmul(
                ph[:], lhsT=w1sb[:, t * H2 : (t + 1) * H2], rhs=int_sb[:],
                start=True, stop=True,
            )
            nc.scalar.activation(h1t[:], ph[:], AF.Tanh, bias=b1sb[:, t : t + 1])
        ph2 = ptrans.tile([P, IC], F32, tag="tr")
        nc.tensor.matmul(ph2[:], lhsT=w2sb[:, 0:H2], rhs=h1a[:], start=True, stop=False)
        nc.tensor.matmul(
            ph2[:], lhsT=w2sb[:, H2 : 2 * H2], rhs=h1b[:], start=False, stop=True
        )
        h2sb = mlp.tile([P, IC], F32, tag="h2")
        nc.scalar.activation(h2sb[:], ph2[:], AF.Tanh, bias=b2sb[:, 0:1])
        pf_ = ptrans.tile([H3, IC], F32, tag="tr")
        nc.tensor.matmul(pf_[:], lhsT=w3sb[:], rhs=h2sb[:], start=True, stop=True)
        nc.scalar.activation(featsF32[0:H3, isl], pf_[:], AF.Sigmoid, bias=b3sb[:, 0:1])

    # fref = MLP(0) via [*,1] column pipeline (tiny)
    for t in range(2):
        nc.scalar.activation(h1ref[:, t : t + 1], zin[:], AF.Tanh, bias=b1sb[:, t : t + 1])
    ph2r = ptrans.tile([P, 1], F32, tag="tr")
    nc.tensor.matmul(ph2r[:], lhsT=w2sb[:, 0:H2], rhs=h1ref[:, 0:1], start=True, stop=False)
    nc.tensor.matmul(ph2r[:], lhsT=w2sb[:, H2 : 2 * H2], rhs=h1ref[:, 1:2], start=False, stop=True)
    nc.scalar.activation(h2ref[:], ph2r[:], AF.Tanh, bias=b2sb[:, 0:1])
    pfr = ptrans.tile([H3, 1], F32, tag="tr")
    nc.tensor.matmul(pfr[:], lhsT=w3sb[:], rhs=h2ref[:], start=True, stop=True)
    nc.scalar.activation(fref[:], pfr[:], AF.Sigmoid, bias=b3sb[:, 0:1])

    # feats output (transposed block, full fp32)
    nc.sync.dma_start(T["featsT"][:], featsF32[:])
    # bf16 residuals: the AG payload (quarters collective bytes vs fp32;
    # ~6e-7 abs rounding on a ~3e-4 signal washes out in the 8192-term sum)
    nc.vector.tensor_scalar(resid16[:], featsF32[:], fref[:], None, op0=ALU.subtract)

    # ---------------- all-gather featsT blocks (fp32) ----------------
    agin = dram.tile([H3, BLK], BF16)
    agout = dram.tile([H3 * NCORES, BLK], BF16, addr_space="Shared")
    nc.sync.dma_start(agin[:], resid16[:])
    nc.gpsimd.collective_compute(
        "AllGather",
        ALU.bypass,
        replica_groups=[list(range(NCORES))],
        ins=[agin.opt()],
        outs=[agout.opt()],
    )

    # centered local features -> augR rows 0..63 (fp32 -> fp32r rounding)
    nc.vector.tensor_scalar(
        augR[0:H3, :], featsF32[:], fref[:], None, op0=ALU.subtract
    )
    # csq_loc at partition 64 (legal compute-engine partition base)
    for ic in range(N_IC):
        isl = slice(ic * IC, (ic + 1) * IC)
        fsqr = fsqp.tile([H3, IC], F32R, tag="fsq")
        nc.scalar.activation(fsqr[:], augR[0:H3, isl], AF.Square)
        psq = psqp.tile([1, IC], F32, tag="sq")
        nc.tensor.matmul(psq[:], lhsT=ones64[:], rhs=fsqr[:], start=True, stop=True)
        nc.scalar.activation(augR[H3 : H3 + 1, isl], psq[:], AF.Copy)
    # ones at partition 65: compute engines can't address base 65; DMA can
    nc.sync.dma_start(augR[H3 + 1 : H3 + 2, :], T["cones"][0:1, :])

    # per-rank augL blocks: rows 0..63 = -2*resid; row 64 = ones; row 65 = csq.
    # Separate tiles keep the d2 matmuls for rank r unblocked as soon as
    # block r is assembled (no false dependency on later blocks).
    for r in range(NCORES):
        aL = augLs[r]
        gblk = gp.tile([H3, BLK], BF16, tag="gblk")
        nc.sync.dma_start(gblk[:], agout[H3 * r : H3 * (r + 1), :])
        nc.vector.tensor_scalar_mul(aL[0:H3, :], gblk[:], -2.0)
        nc.sync.dma_start(aL[H3 : H3 + 1, :], T["cones"][0:1, :])
        for g in range(N_IC):
            gsl = slice(g * IC, (g + 1) * IC)
            fsql = fsqp.tile([H3, IC], F32R, tag="fsq")
            nc.scalar.activation(fsql[:], aL[0:H3, gsl], AF.Square)  # = 4*cfeat^2
            psql = psqp.tile([1, IC], F32, tag="sq")
            nc.tensor.matmul(psql[:], lhsT=ones64[:], rhs=fsql[:], start=True, stop=True)
            sqst = fsqp.tile([1, IC], F32R, tag="fsq")
            nc.scalar.activation(sqst[:], psql[:], AF.Copy, scale=0.25)
            nc.sync.dma_start(aL[H3 + 1 : H3 + 2, gsl], sqst[:])

    # ---------------- phase 2: dist block, contract with members ----------------
    pout0 = pacc.tile([NCLUST, IC], F32, tag="acc")
    pout1 = pacc.tile([NCLUST, IC], F32, tag="acc")
    pouts = [pout0, pout1]
    # dist = sqrt(d2 + 2e-10): the bias keeps the (centered, ~1e-11-noise)
    # diagonal non-negative, replacing the reference's max(d2, 1e-12) clamp
    # with O(1e-4) relative effect only on the closest pair's distance.
    for jp in range(N_K // 2):
        pft2 = pfp.tile([P, 2 * BLK], PF_DT, tag="pft")
        nc.sync.dma_start(
            pft2[:].rearrange("p (t c) -> p t c", c=BLK),
            T["pf2t"][2 * jp * P : (2 * jp + 2) * P, :].rearrange(
                "(t p) c -> p t c", p=P
            ),
        )
        for jh in range(2):
            jc = 2 * jp + jh
            for ic in range(N_IC):
                isl = slice(ic * IC, (ic + 1) * IC)
                pd2 = ptrans.tile([P, IC], F32, tag="tr")
                nc.tensor.matmul(
                    pd2[:],
                    lhsT=augLs[jc // 8][:, (jc % 8) * P : (jc % 8 + 1) * P],
                    rhs=augR[:, isl],
                    start=True,
                    stop=True,
                )
                tt = fwork.tile([P, IC], F32, tag="tt")
                nc.vector.scalar_tensor_tensor(
                    tt[:],
                    pd2[:],
                    1e-12,
                    pft2[:, jh * BLK + ic * IC : jh * BLK + (ic + 1) * IC],
                    op0=ALU.max,
                    op1=ALU.mult,
                )
                fo = fwork.tile([P, IC], F32R, tag="fo")
                nc.scalar.activation(fo[:], tt[:], AF.Sqrt)
                nc.tensor.matmul(
                    pouts[ic][:],
                    lhsT=memsb[:, jc * NCLUST : (jc + 1) * NCLUST],
                    rhs=fo[:],
                    start=(jc == 0),
                    stop=(jc == N_K - 1),
                )
    for ic in range(N_IC):
        osb = fwork.tile([NCLUST, IC], F32, tag="tt")
        nc.vector.tensor_copy(osb[:], pouts[ic][:])
        nc.sync.dma_start(T["outT"][:, ic * IC : (ic + 1) * IC], osb[:])

    for p in reversed(ctxpools):
        p.__exit__(None, None, None)


def _build():
    nc = bacc.Bacc(
        "TRN2",
        target_bir_lowering=False,
        debug=False,
        enable_asserts=False,
        num_devices=NCORES,
    )
    T = {}
    T["fthl"] = nc.dram_tensor("fthl", [N, 2 * BLK], BF16, kind="ExternalInput").ap()
    T["xhi"] = nc.dram_tensor("xhi", [P, N_K * IN_F], BF16, kind="ExternalInput").ap()
    T["xlo"] = nc.dram_tensor("xlo", [P, N_K * IN_F], BF16, kind="ExternalInput").ap()
    T["pf2t"] = nc.dram_tensor("pf2t", [N, BLK], PF_DT, kind="ExternalInput").ap()
    T["members"] = nc.dram_tensor("members", [P, N_K * NCLUST], F32R, kind="ExternalInput").ap()
    T["w1"] = nc.dram_tensor("w1", [IN_F, H1], F32, kind="ExternalInput").ap()
    T["b1"] = nc.dram_tensor("b1", [H1], F32, kind="ExternalInput").ap()
    T["w2"] = nc.dram_tensor("w2", [H1, H2], F32, kind="ExternalInput").ap()
    T["b2"] = nc.dram_tensor("b2", [H2], F32, kind="ExternalInput").ap()
    T["w3"] = nc.dram_tensor("w3", [H2, H3], F32, kind="ExternalInput").ap()
    T["b3"] = nc.dram_tensor("b3", [H3], F32, kind="ExternalInput").ap()
    T["cones"] = nc.dram_tensor("cones", [H3, BLK], F32R, kind="ExternalInput").ap()
    T["featsT"] = nc.dram_tensor("featsT", [H3, BLK], F32, kind="ExternalOutput").ap()
    T["outT"] = nc.dram_tensor("outT", [NCLUST, BLK], F32, kind="ExternalOutput").ap()
    with tile.TileContext(nc) as tc:
        _emit(tc, nc, T)
    nc.compile()
    return nc


_NC = None
_ONES = np.ones((H3, BLK), np.float32)
_PF_NP = mybir.dt.np(PF_DT)


def _get_nc():
    global _NC
    if _NC is None:
        _NC = _build()
    return _NC


def kernel(F_, X, path_forces, members, W1, b1, W2, b2, W3, b3,
           _trace=False, _return_raw=False, _tmpdir=None):
    F_ = np.asarray(F_, np.float32)
    X = np.asarray(X, np.float32)
    path_forces = np.asarray(path_forces, np.float32)
    members = np.asarray(members, np.float32)
    W1 = np.asarray(W1, np.float32)
    b1 = np.asarray(b1, np.float32)
    W2 = np.asarray(W2, np.float32)
    b2 = np.asarray(b2, np.float32)
    W3 = np.asarray(W3, np.float32)
    b3 = np.asarray(b3, np.float32)

    nc = _get_nc()

    bf = ml_dtypes.bfloat16

    def _prearrange(a):  # [64*128, C] -> [128, 64*C]
        c = a.shape[1]
        return np.ascontiguousarray(
            a.reshape(N_K, P, c).transpose(1, 0, 2).reshape(P, N_K * c)
        )

    xhi32 = X.astype(bf).astype(np.float32)
    xhi = _prearrange(X.astype(bf).astype(np.float32)).astype(bf)
    xlo = _prearrange(X - xhi32).astype(bf)
    members_pa = _prearrange(members)
    pf2 = path_forces * path_forces
    np.fill_diagonal(pf2, 0.0)

    in_maps = []
    for c in range(NCORES):
        sl = slice(c * BLK, (c + 1) * BLK)
        ftT = np.ascontiguousarray(F_[sl, :].T)
        fthi = ftT.astype(bf)
        ftlo = (ftT - fthi.astype(np.float32)).astype(bf)
        fthl = np.concatenate([fthi, ftlo], axis=1)
        in_maps.append(
            {
                "fthl": fthl,
                "xhi": xhi,
                "xlo": xlo,
                "pf2t": np.ascontiguousarray(pf2[sl, :].T).astype(_PF_NP),
                "members": members_pa,
                "w1": W1,
                "b1": b1,
                "w2": W2,
                "b2": b2,
                "w3": W3,
                "b3": b3,
                "cones": _ONES,
            }
        )

    res = bass_utils.run_bass_kernel_spmd(
        nc, in_maps, core_ids=list(range(NCORES)), trace=_trace, tmpdir=_tmpdir
    )
    feats = np.concatenate(
        [np.asarray(res.results[c]["featsT"]).T for c in range(NCORES)], axis=0
    )
    out = np.concatenate(
        [np.asarray(res.results[c]["outT"]).T for c in range(NCORES)], axis=0
    )
    if _return_raw:
        return (feats, out), res
    return feats, out
